# revision 13
# baseline (speedup 1.0000x reference)
"""Single-head attention (B=4, T=4096, C=1024, H=64) on 8 trn2 NeuronCores.

Sharding: 8 shards = (batch b, query-half h).  Each core receives x[b]
pre-transposed to xT [C=1024, T=4096]; for h==1 the T columns are rotated by
2048 so that "this core's" 2048 queries are always columns 0:2048 (softmax is
permutation-invariant over keys).  SPMD: identical program on every core.

v3 (bf16 + row-tiled scores + two-engine exp):
  All matmul inputs are bf16 (1 cyc/row on the PE; half the HBM traffic).

  Scores contract over H=64, which would idle half the PE array.  Instead,
  key tiles are processed in PAIRS via PE row tiling: K^T for one tile of
  the pair lives in SBUF partitions 0:64, the other in 64:128 (arranged by
  swapping the projection stationary to [Wv|Wk'] for alternate 1024-column
  groups of x, so both layouts fall out of the same matmul), and Q^T is
  duplicated into partitions 64:128 ([Wq|Wq] stationary).  The two score
  matmuls then run CONCURRENTLY on disjoint row groups of the array --
  2x on the score stage.  Pairs are formed within each 2048-column x half
  so early attention only depends on the first half of the x stream.

  The exp stage (8.4M elements; ScalarE alone would take ~70us) is split
  across two engines via the Schraudolph bit trick: scores reach PSUM
  pre-scaled by 16*log2(e) (folded into Wk host-side), so
  int16(s + 16248.5) reinterpreted as bf16 IS exp(0.125*s_qk)*(1+-3%).
  DVE computes its share of exp with a single tensor_scalar_add
  (f32 PSUM -> int16-bitcast-bf16 SBUF); ACT computes exact exp for its
  share (scale un-maps the fold; the +-3% chord error of the DVE share
  washes out in the softmax average).  One [128,1024] op per key pair.

  attn@V consumes ex bf16 against V packed [s,64|ones] (the ones column
  yields the softmax denominator from the same accumulation); the PSUM
  [65,512] result is PE-transposed back to [t,65], scaled by the
  reciprocal of the denominator column, and DMA'd out in fp32.
"""

import os
import sys

for _p in ("/opt/trn_rl_repo", "/root/.axon_site/_ro/trn_rl_repo"):
    if os.path.isdir(_p) and _p not in sys.path:
        sys.path.append(_p)

import numpy as np
import ml_dtypes

import concourse.bacc as bacc
import concourse.mybir as mybir
import concourse.tile as tile
from concourse.bass_utils import run_bass_kernel_spmd
from concourse.masks import make_identity

B = 4
T = 4096
C = 1024
H = 64
TQ = T // 2  # queries per core
N_CORES = 8

F32 = mybir.dt.float32
BF16 = mybir.dt.bfloat16
I16 = mybir.dt.int16

NC_CH = C // 128  # 8 contraction chunks
NBLK = T // 512  # 8 projection blocks of 512
NPAIR = T // 256  # 16 key pair-blocks of 256
NST = T // 128  # 32 key tiles
NTC = TQ // 512  # 4 query chunks of 512

# exp(0.125*s) ~= bits_as_bf16(trunc(23.083*s + 16248.5))
K_FOLD = 0.125 * 128.0 / np.log(2.0)  # 23.0831
SCH_B = 16248.5  # 2^7 * (127 - 0.0586)
ACT_SCALE = float(np.log(2.0) / 128.0)

EXP = mybir.ActivationFunctionType.Exp
COPY = mybir.ActivationFunctionType.Copy


def _build_module():
    nc = bacc.Bacc("TRN2", target_bir_lowering=False, debug=False, num_devices=N_CORES)

    xT = nc.dram_tensor("xT", [NC_CH, 128, T], BF16, kind="ExternalInput").ap()
    # [:, c, 0:128] = [Wk'|Wv] chunk; [:, c, 128:256] = [Wv|Wk'] chunk
    wkv = nc.dram_tensor("wkv", [128, NC_CH, 256], BF16, kind="ExternalInput").ap()
    wq2 = nc.dram_tensor("wq2", [128, NC_CH, 128], BF16, kind="ExternalInput").ap()
    out = nc.dram_tensor("out", [TQ, H], F32, kind="ExternalOutput").ap()

    with tile.TileContext(nc) as tc:
        with (
            tc.tile_pool(name="const", bufs=1) as const_pool,
            tc.tile_pool(name="xt", bufs=16) as xt_pool,
            tc.tile_pool(name="big", bufs=1) as big_pool,
            tc.tile_pool(name="exp", bufs=6) as exp_pool,
            tc.tile_pool(name="outts", bufs=2) as outts_pool,
            tc.tile_pool(name="small", bufs=4) as small_pool,
            tc.tile_pool(name="p1", bufs=2, space="PSUM") as psum_p1,
            tc.tile_pool(name="psc", bufs=2, space="PSUM") as psum_sc,
            tc.tile_pool(name="pacc", bufs=2, space="PSUM") as psum_acc,
        ):
            # ---- constants ----
            wkv_sb = const_pool.tile([128, NC_CH, 256], BF16, tag="wkv")
            wq_sb = const_pool.tile([128, NC_CH, 128], BF16, tag="wq")
            ident_bf = const_pool.tile([128, 128], BF16, tag="ident_bf")
            ident_f32 = const_pool.tile([65, 65], F32, tag="ident_f32")
            scratch = const_pool.tile([128, 1], F32, tag="scratch")
            nc.sync.dma_start(wkv_sb[:], wkv)
            nc.sync.dma_start(wq_sb[:], wq2)
            make_identity(nc, ident_bf[:])
            make_identity(nc, ident_f32[:])
            # pull the exp table load off the critical path
            nc.scalar.activation(scratch[:], ident_bf[:, 0:1], EXP)

            # ---- persistent activations ----
            # kt2 col j*128+i: rows 0:64 = K' of pair-j's A key tile,
            # rows 64:128 = K' of its B key tile (A/B = the two 1024-col
            # groups of the same x half)
            kt2 = big_pool.tile([128, 2048], BF16, tag="kt2")
            vt2 = big_pool.tile([128, 2048], BF16, tag="vt2")
            qt2 = big_pool.tile([128, TQ], BF16, tag="qt2")  # Q^T duplicated
            va = big_pool.tile([128, NST, 65], BF16, tag="va")  # V | ones col
            nc.gpsimd.memset(va[:, :, 64:65], 1.0)

            # ---- x DMA (half 0 first; queues rotate) ----
            dma_engines = (nc.sync, nc.gpsimd, nc.scalar)
            xts = {}
            for half in range(2):
                for c in range(NC_CH):
                    xt_t = xt_pool.tile([128, 2048], BF16, tag="xt")
                    dma_engines[(half * NC_CH + c) % 3].dma_start(
                        xt_t[:], xT[c, :, half * 2048 : (half + 1) * 2048]
                    )
                    xts[(half, c)] = xt_t

            # ---- phase 1: projections per 512-col block ----
            def emit_proj_block(sb):
                half, off = divmod(sb * 512, 2048)
                sw = (sb // 2) % 2  # 1 -> [Wv|Wk'] (K lands in rows 64:128)
                kbase = (sb // 4) * 1024 + (sb % 2) * 512
                kcol = slice(kbase, kbase + 512)
                kv_ps = psum_p1.tile([128, 512], F32, tag="p1")
                for c in range(NC_CH):
                    nc.tensor.matmul(
                        kv_ps[:],
                        wkv_sb[:, c, sw * 128 : sw * 128 + 128],
                        xts[(half, c)][:, off : off + 512],
                        start=(c == 0),
                        stop=(c == NC_CH - 1),
                    )
                if sw == 0:
                    nc.vector.tensor_copy(kt2[0:64, kcol], kv_ps[0:64, :])
                    nc.scalar.activation(vt2[64:128, kcol], kv_ps[64:128, :], COPY)
                else:
                    nc.vector.tensor_copy(kt2[64:128, kcol], kv_ps[64:128, :])
                    nc.scalar.activation(vt2[0:64, kcol], kv_ps[0:64, :], COPY)
                if sb < NTC:  # queries = keys 0:2048
                    q_ps = psum_p1.tile([128, 512], F32, tag="p1")
                    for c in range(NC_CH):
                        nc.tensor.matmul(
                            q_ps[:],
                            wq_sb[:, c, :],
                            xts[(half, c)][:, off : off + 512],
                            start=(c == 0),
                            stop=(c == NC_CH - 1),
                        )
                    nc.vector.tensor_copy(qt2[:, sb * 512 : (sb + 1) * 512], q_ps[:])
                # V^T -> V transposes (key-tile ids in natural key order)
                vabase = (sb // 2) * 8 + (sb % 2) * 4
                for jj in range(4):
                    st = vabase + jj
                    col = slice(kbase + jj * 128, kbase + (jj + 1) * 128)
                    vt_ps = psum_p1.tile([128, 64], BF16, tag="p1")
                    if sw == 0:
                        nc.tensor.transpose(
                            vt_ps[:], vt2[64:128, col], ident_bf[64:128, 64:128]
                        )
                    else:
                        nc.tensor.transpose(
                            vt_ps[:], vt2[0:64, col], ident_bf[0:64, 0:64]
                        )
                    nc.vector.tensor_copy(va[:, st, 0:64], vt_ps[:])

            # ---- phase 2: attention ----
            acc_tiles = {}
            exp_idx = [0]

            def emit_attn(tcp, pair_lo, pair_hi):
                if tcp not in acc_tiles:
                    acc_tiles[tcp] = psum_acc.tile(
                        [65, 512], F32, tag="acc", name=f"av{tcp}"
                    )
                av_ps = acc_tiles[tcp]
                tq = slice(tcp * 512, (tcp + 1) * 512)
                pend = []  # delayed-by-one attn@V emission to keep PE streaming

                def flush():
                    for args, kwargs in pend:
                        nc.tensor.matmul(*args, **kwargs)
                    pend.clear()

                for j in range(pair_lo, pair_hi):
                    jc = slice(j * 128, (j + 1) * 128)
                    sc_ps = psum_sc.tile([128, 1024], F32, tag="sc")
                    nc.tensor.matmul(
                        sc_ps[:, 0:512], kt2[0:64, jc], qt2[0:64, tq],
                        start=True, stop=True,
                    )
                    nc.tensor.matmul(
                        sc_ps[:, 512:1024], kt2[64:128, jc], qt2[64:128, tq],
                        start=True, stop=True,
                    )
                    flush()
                    ex = exp_pool.tile([128, 1024], BF16, tag="exp")
                    if exp_idx[0] % 8 < 5:
                        nc.scalar.activation(ex[:], sc_ps[:], EXP, scale=ACT_SCALE)
                    else:
                        nc.vector.tensor_scalar_add(ex[:].bitcast(I16), sc_ps[:], SCH_B)
                    exp_idx[0] += 1
                    aid = j if j < 8 else j + 8
                    for st, excol in ((aid, slice(0, 512)), (aid + 8, slice(512, 1024))):
                        pend.append(
                            (
                                (av_ps[:], va[:, st, :], ex[:, excol]),
                                dict(start=(st == 0), stop=(st == NST - 1)),
                            )
                        )
                flush()

            def emit_epilogue(tcp):
                av_ps = acc_tiles[tcp]
                outt_sb = outts_pool.tile([65, 512], F32, tag="outts")
                nc.scalar.activation(outt_sb[:], av_ps[:], COPY)
                for j in range(4):
                    o_ps = psum_p1.tile([128, 65], F32, tag="p1")
                    nc.tensor.transpose(
                        o_ps[:], outt_sb[:, j * 128 : (j + 1) * 128], ident_f32[:]
                    )
                    rc = small_pool.tile([128, 1], F32, tag="rc")
                    nc.vector.reciprocal(rc[:], o_ps[:, 64:65])
                    o_sb = small_pool.tile([128, H], F32, tag="osb")
                    nc.vector.tensor_scalar_mul(o_sb[:], o_ps[:, 0:H], rc[:])
                    row = tcp * 512 + j * 128
                    dma_engines[j % 2].dma_start(out[row : row + 128, :], o_sb[:])

            # emission order: half-0 projections; attention pairs 0:8 of two
            # query chunks (they only need half 0) overlap the half-1 x DMA.
            for sb in range(4):
                emit_proj_block(sb)
            emit_attn(0, 0, 8)
            emit_attn(1, 0, 8)
            for sb in range(4, NBLK):
                emit_proj_block(sb)
            emit_attn(0, 8, NPAIR)
            emit_epilogue(0)
            emit_attn(1, 8, NPAIR)
            emit_epilogue(1)
            for tcp in range(2, NTC):
                emit_attn(tcp, 0, NPAIR)
                emit_epilogue(tcp)

    nc.compile()
    return nc


_NC_CACHE = None


def _get_module():
    global _NC_CACHE
    if _NC_CACHE is None:
        _NC_CACHE = _build_module()
    return _NC_CACHE


def _make_in_maps(x, Wq, Wk, Wv):
    x64 = np.asarray(x, dtype=np.float64)
    wq64 = np.asarray(Wq, dtype=np.float64)
    wk64 = np.asarray(Wk, dtype=np.float64) * K_FOLD
    wv64 = np.asarray(Wv, dtype=np.float64)
    wkv64 = np.concatenate([wk64, wv64, wv64, wk64], axis=1)  # [C, 256]
    wkv_t = np.ascontiguousarray(
        wkv64.reshape(NC_CH, 128, 256).transpose(1, 0, 2)
    ).astype(ml_dtypes.bfloat16)
    wq2_t = np.ascontiguousarray(
        np.concatenate([wq64, wq64], axis=1).reshape(NC_CH, 128, 128).transpose(1, 0, 2)
    ).astype(ml_dtypes.bfloat16)
    in_maps = []
    for core in range(N_CORES):
        b, h = divmod(core, 2)
        xt = x64[b].T  # [C, T]
        if h == 1:
            xt = np.concatenate([xt[:, TQ:], xt[:, :TQ]], axis=1)
        xt = np.ascontiguousarray(xt.reshape(NC_CH, 128, T)).astype(ml_dtypes.bfloat16)
        in_maps.append({"xT": xt, "wkv": wkv_t, "wq2": wq2_t})
    return in_maps


def run(x, Wq, Wk, Wv, **spmd_kwargs):
    """Run on hardware; returns (output, BassKernelResults)."""
    nc = _get_module()
    in_maps = _make_in_maps(x, Wq, Wk, Wv)
    res = run_bass_kernel_spmd(nc, in_maps, core_ids=list(range(N_CORES)), **spmd_kwargs)
    out = np.empty((B, T, H), dtype=np.float32)
    for core in range(N_CORES):
        b, h = divmod(core, 2)
        out[b, h * TQ : (h + 1) * TQ, :] = res.results[core]["out"]
    return out, res


def kernel(x, Wq, Wk, Wv):
    out, _ = run(x, Wq, Wk, Wv)
    return out


# revision 15
# speedup vs baseline: 1.2297x; 1.2297x over previous
"""Single-head attention (B=4, T=4096, C=1024, H=64) on 8 trn2 NeuronCores.

Sharding: 8 shards = (batch b, query-half h).  Each core receives x[b]
pre-transposed to xT [C=1024, T=4096]; for h==1 the T columns are rotated by
2048 so that "this core's" 2048 queries are always columns 0:2048 (softmax is
permutation-invariant over keys).  SPMD: identical program on every core.

v3 (bf16 + row-tiled scores + two-engine exp):
  All matmul inputs are bf16 (1 cyc/row on the PE; half the HBM traffic).

  Scores contract over H=64, which would idle half the PE array.  Instead,
  key tiles are processed in PAIRS via PE row tiling: K^T for one tile of
  the pair lives in SBUF partitions 0:64, the other in 64:128 (arranged by
  swapping the projection stationary to [Wv|Wk'] for alternate 1024-column
  groups of x, so both layouts fall out of the same matmul), and Q^T is
  duplicated into partitions 64:128 ([Wq|Wq] stationary).  The two score
  matmuls then run CONCURRENTLY on disjoint row groups of the array --
  2x on the score stage.  Pairs are formed within each 2048-column x half
  so early attention only depends on the first half of the x stream.

  The exp stage (8.4M elements; ScalarE alone would take ~70us) is split
  across two engines via the Schraudolph bit trick: scores reach PSUM
  pre-scaled by 16*log2(e) (folded into Wk host-side), so
  int16(s + 16248.5) reinterpreted as bf16 IS exp(0.125*s_qk)*(1+-3%).
  DVE computes its share of exp with a single tensor_scalar_add
  (f32 PSUM -> int16-bitcast-bf16 SBUF); ACT computes exact exp for its
  share (scale un-maps the fold; the +-3% chord error of the DVE share
  washes out in the softmax average).  One [128,1024] op per key pair.

  attn@V consumes ex bf16 against V packed [s,64|ones] (the ones column
  yields the softmax denominator from the same accumulation); the PSUM
  [65,512] result is PE-transposed back to [t,65], scaled by the
  reciprocal of the denominator column, and DMA'd out in fp32.
"""

import os
import sys

for _p in ("/opt/trn_rl_repo", "/root/.axon_site/_ro/trn_rl_repo"):
    if os.path.isdir(_p) and _p not in sys.path:
        sys.path.append(_p)

import numpy as np
import ml_dtypes

import concourse.bacc as bacc
import concourse.mybir as mybir
import concourse.tile as tile
from concourse.bass_utils import run_bass_kernel_spmd
from concourse.masks import make_identity

B = 4
T = 4096
C = 1024
H = 64
TQ = T // 2  # queries per core
N_CORES = 8

F32 = mybir.dt.float32
BF16 = mybir.dt.bfloat16
I16 = mybir.dt.int16

NC_CH = C // 128  # 8 contraction chunks
NBLK = T // 512  # 8 projection blocks of 512
NPAIR = T // 256  # 16 key pair-blocks of 256
NST = T // 128  # 32 key tiles
NTC = TQ // 512  # 4 query chunks of 512

# exp(0.125*s) ~= bits_as_bf16(trunc(23.083*s + 16248.5))
K_FOLD = 0.125 * 128.0 / np.log(2.0)  # 23.0831
SCH_B = 16248.5  # 2^7 * (127 - 0.0586)
ACT_SCALE = float(np.log(2.0) / 128.0)

EXP = mybir.ActivationFunctionType.Exp
COPY = mybir.ActivationFunctionType.Copy


def _build_module():
    nc = bacc.Bacc("TRN2", target_bir_lowering=False, debug=False, num_devices=N_CORES)

    xT = nc.dram_tensor("xT", [NC_CH, 128, T], BF16, kind="ExternalInput").ap()
    # [:, c, 0:128] = [Wk'|Wv] chunk; [:, c, 128:256] = [Wv|Wk'] chunk
    wkv = nc.dram_tensor("wkv", [128, NC_CH, 256], BF16, kind="ExternalInput").ap()
    wq2 = nc.dram_tensor("wq2", [128, NC_CH, 128], BF16, kind="ExternalInput").ap()
    out = nc.dram_tensor("out", [TQ, H], F32, kind="ExternalOutput").ap()

    with tile.TileContext(nc) as tc:
        with (
            tc.tile_pool(name="const", bufs=1) as const_pool,
            tc.tile_pool(name="xt", bufs=16) as xt_pool,
            tc.tile_pool(name="big", bufs=1) as big_pool,
            tc.tile_pool(name="exp", bufs=6) as exp_pool,
            tc.tile_pool(name="outts", bufs=2) as outts_pool,
            tc.tile_pool(name="small", bufs=4) as small_pool,
            tc.tile_pool(name="p1", bufs=2, space="PSUM") as psum_p1,
            tc.tile_pool(name="psc", bufs=2, space="PSUM") as psum_sc,
            tc.tile_pool(name="pacc", bufs=2, space="PSUM") as psum_acc,
        ):
            # ---- constants ----
            wkv_sb = const_pool.tile([128, NC_CH, 256], BF16, tag="wkv")
            wq_sb = const_pool.tile([128, NC_CH, 128], BF16, tag="wq")
            ident_bf = const_pool.tile([128, 128], BF16, tag="ident_bf")
            ident_f32 = const_pool.tile([65, 65], F32, tag="ident_f32")
            scratch = const_pool.tile([128, 1], F32, tag="scratch")
            nc.sync.dma_start(wkv_sb[:], wkv)
            nc.sync.dma_start(wq_sb[:], wq2)
            make_identity(nc, ident_bf[:])
            make_identity(nc, ident_f32[:])
            # pull the exp table load off the critical path
            nc.scalar.activation(scratch[:], ident_bf[:, 0:1], EXP)

            # ---- persistent activations ----
            # kt2 col j*128+i: rows 0:64 = K' of pair-j's A key tile,
            # rows 64:128 = K' of its B key tile (A/B = the two 1024-col
            # groups of the same x half)
            kt2 = big_pool.tile([128, 2048], BF16, tag="kt2")
            vt2 = big_pool.tile([128, 2048], BF16, tag="vt2")
            qt2 = big_pool.tile([128, TQ], BF16, tag="qt2")  # Q^T duplicated
            va = big_pool.tile([128, NST, 65], BF16, tag="va")  # V | ones col
            nc.gpsimd.memset(va[:, :, 64:65], 1.0)

            # ---- x DMA (half 0 first; queues rotate) ----
            dma_engines = (nc.sync, nc.gpsimd, nc.scalar)
            xts = {}
            for half in range(2):
                for c in range(NC_CH):
                    xt_t = xt_pool.tile([128, 2048], BF16, tag="xt")
                    dma_engines[(half * NC_CH + c) % 3].dma_start(
                        xt_t[:], xT[c, :, half * 2048 : (half + 1) * 2048]
                    )
                    xts[(half, c)] = xt_t

            # ---- phase 1: projections per 512-col block ----
            def emit_proj_block(sb):
                half, off = divmod(sb * 512, 2048)
                sw = (sb // 2) % 2  # 1 -> [Wv|Wk'] (K lands in rows 64:128)
                kbase = (sb // 4) * 1024 + (sb % 2) * 512
                kcol = slice(kbase, kbase + 512)
                kv_ps = psum_p1.tile([128, 512], F32, tag="p1")
                for c in range(NC_CH):
                    nc.tensor.matmul(
                        kv_ps[:],
                        wkv_sb[:, c, sw * 128 : sw * 128 + 128],
                        xts[(half, c)][:, off : off + 512],
                        start=(c == 0),
                        stop=(c == NC_CH - 1),
                    )
                if sw == 0:
                    nc.vector.tensor_copy(kt2[0:64, kcol], kv_ps[0:64, :])
                    nc.scalar.activation(vt2[64:128, kcol], kv_ps[64:128, :], COPY)
                else:
                    nc.vector.tensor_copy(kt2[64:128, kcol], kv_ps[64:128, :])
                    nc.scalar.activation(vt2[0:64, kcol], kv_ps[0:64, :], COPY)
                if sb < NTC:  # queries = keys 0:2048
                    q_ps = psum_p1.tile([128, 512], F32, tag="p1")
                    for c in range(NC_CH):
                        nc.tensor.matmul(
                            q_ps[:],
                            wq_sb[:, c, :],
                            xts[(half, c)][:, off : off + 512],
                            start=(c == 0),
                            stop=(c == NC_CH - 1),
                        )
                    nc.vector.tensor_copy(qt2[:, sb * 512 : (sb + 1) * 512], q_ps[:])
                # V^T -> V transposes (key-tile ids in natural key order)
                vabase = (sb // 2) * 8 + (sb % 2) * 4
                for jj in range(4):
                    st = vabase + jj
                    col = slice(kbase + jj * 128, kbase + (jj + 1) * 128)
                    vt_ps = psum_p1.tile([128, 64], BF16, tag="p1")
                    if sw == 0:
                        nc.tensor.transpose(
                            vt_ps[:], vt2[64:128, col], ident_bf[64:128, 64:128]
                        )
                    else:
                        nc.tensor.transpose(
                            vt_ps[:], vt2[0:64, col], ident_bf[0:64, 0:64]
                        )
                    nc.vector.tensor_copy(va[:, st, 0:64], vt_ps[:])

            # ---- phase 2: attention ----
            acc_tiles = {}
            exp_idx = [0]

            def emit_attn(tcp, pair_lo, pair_hi):
                if tcp not in acc_tiles:
                    acc_tiles[tcp] = psum_acc.tile(
                        [65, 512], F32, tag="acc", name=f"av{tcp}"
                    )
                av_ps = acc_tiles[tcp]
                tq = slice(tcp * 512, (tcp + 1) * 512)
                pend = []  # delayed-by-two attn@V emission to keep PE streaming

                def flush(keep=0):
                    while len(pend) > keep:
                        args, kwargs = pend.pop(0)
                        nc.tensor.matmul(*args, **kwargs)

                for j in range(pair_lo, pair_hi):
                    jc = slice(j * 128, (j + 1) * 128)
                    sc_ps = psum_sc.tile([128, 1024], F32, tag="sc")
                    nc.tensor.matmul(
                        sc_ps[:, 0:512], kt2[0:64, jc], qt2[0:64, tq],
                        start=True, stop=True,
                    )
                    nc.tensor.matmul(
                        sc_ps[:, 512:1024], kt2[64:128, jc], qt2[64:128, tq],
                        start=True, stop=True,
                    )
                    flush(keep=4)  # av MMs trail the scores by two pairs
                    ex = exp_pool.tile([128, 1024], BF16, tag="exp")
                    i = exp_idx[0]
                    if (i * 17) % 32 < 17:  # ~17/32 of pairs on ACT
                        nc.scalar.activation(ex[:], sc_ps[:], EXP, scale=ACT_SCALE)
                    else:
                        nc.vector.tensor_scalar_add(ex[:].bitcast(I16), sc_ps[:], SCH_B)
                    exp_idx[0] += 1
                    aid = j if j < 8 else j + 8
                    for st, excol in ((aid, slice(0, 512)), (aid + 8, slice(512, 1024))):
                        pend.append(
                            (
                                (av_ps[:], va[:, st, :], ex[:, excol]),
                                dict(start=(st == 0), stop=(st == NST - 1)),
                            )
                        )
                flush()

            def emit_epilogue(tcp):
                av_ps = acc_tiles[tcp]
                outt_sb = outts_pool.tile([65, 512], F32, tag="outts")
                nc.scalar.activation(outt_sb[:], av_ps[:], COPY)
                for j in range(4):
                    o_ps = psum_p1.tile([128, 65], F32, tag="p1")
                    nc.tensor.transpose(
                        o_ps[:], outt_sb[:, j * 128 : (j + 1) * 128], ident_f32[:]
                    )
                    rc = small_pool.tile([128, 1], F32, tag="rc")
                    nc.vector.reciprocal(rc[:], o_ps[:, 64:65])
                    o_sb = small_pool.tile([128, H], F32, tag="osb")
                    nc.vector.tensor_scalar_mul(o_sb[:], o_ps[:, 0:H], rc[:])
                    row = tcp * 512 + j * 128
                    dma_engines[j % 2].dma_start(out[row : row + 128, :], o_sb[:])

            # emission order: half-0 projections; attention pairs 0:8 of two
            # query chunks (they only need half 0) overlap the half-1 x DMA.
            for sb in range(4):
                emit_proj_block(sb)
            emit_attn(0, 0, 8)
            emit_attn(1, 0, 8)
            for sb in range(4, NBLK):
                emit_proj_block(sb)
            emit_attn(0, 8, NPAIR)
            emit_epilogue(0)
            emit_attn(1, 8, NPAIR)
            emit_epilogue(1)
            for tcp in range(2, NTC):
                emit_attn(tcp, 0, NPAIR)
                emit_epilogue(tcp)

    nc.compile()
    return nc


_NC_CACHE = None


def _get_module():
    global _NC_CACHE
    if _NC_CACHE is None:
        _NC_CACHE = _build_module()
    return _NC_CACHE


def _make_in_maps(x, Wq, Wk, Wv):
    x64 = np.asarray(x, dtype=np.float64)
    wq64 = np.asarray(Wq, dtype=np.float64)
    wk64 = np.asarray(Wk, dtype=np.float64) * K_FOLD
    wv64 = np.asarray(Wv, dtype=np.float64)
    wkv64 = np.concatenate([wk64, wv64, wv64, wk64], axis=1)  # [C, 256]
    wkv_t = np.ascontiguousarray(
        wkv64.reshape(NC_CH, 128, 256).transpose(1, 0, 2)
    ).astype(ml_dtypes.bfloat16)
    wq2_t = np.ascontiguousarray(
        np.concatenate([wq64, wq64], axis=1).reshape(NC_CH, 128, 128).transpose(1, 0, 2)
    ).astype(ml_dtypes.bfloat16)
    in_maps = []
    for core in range(N_CORES):
        b, h = divmod(core, 2)
        xt = x64[b].T  # [C, T]
        if h == 1:
            xt = np.concatenate([xt[:, TQ:], xt[:, :TQ]], axis=1)
        xt = np.ascontiguousarray(xt.reshape(NC_CH, 128, T)).astype(ml_dtypes.bfloat16)
        in_maps.append({"xT": xt, "wkv": wkv_t, "wq2": wq2_t})
    return in_maps


def run(x, Wq, Wk, Wv, **spmd_kwargs):
    """Run on hardware; returns (output, BassKernelResults)."""
    nc = _get_module()
    in_maps = _make_in_maps(x, Wq, Wk, Wv)
    res = run_bass_kernel_spmd(nc, in_maps, core_ids=list(range(N_CORES)), **spmd_kwargs)
    out = np.empty((B, T, H), dtype=np.float32)
    for core in range(N_CORES):
        b, h = divmod(core, 2)
        out[b, h * TQ : (h + 1) * TQ, :] = res.results[core]["out"]
    return out, res


def kernel(x, Wq, Wk, Wv):
    out, _ = run(x, Wq, Wk, Wv)
    return out
